# revision 13
# baseline (speedup 1.0000x reference)
"""Trainium2 Bass kernel for nn_AttentionLayer (single-query GQA cross-attention
+ SwiGLU MLP + residual LayerNorm), SPMD over 8 NeuronCores.

Math (per batch row b, reference-equivalent by associativity):
    q[h]      = Wq_blk(h) @ dec[b]                      (fp32)
    a[h]      = Wk_blk(h//2).T @ q[h] / 8               (fp32, evicted f32r)
    scores[h] = enc_b @ a[h]                            (f32r, tiled over t)
    p         = exp(scores)  (no max-sub: |scores| < ~3), l = sum_t p
    u[h]      = (p[h] @ enc_b) / l[h]                   (f32r accum in PSUM)
    ctx[h]    = Wv_blk(h//2) @ u[h]                     (fp32)
    h_cat     = [ctx, dec];  MLP tensor-parallel over the 4096 hidden dim:
    out       = LayerNorm(silu(h_cat @ W1.T) @ W2.T + dec) (AllReduce partials)

Sharding: batch-parallel attention (4 rows/core), tensor-parallel MLP
(AllGather ctx [tiny] + AllReduce combined [tiny]).  The 536 MB encoder read
(67 MB/core) is the roofline.
"""

import copy
import os
import sys
import types
from contextlib import ExitStack

import numpy as np

sys.path.insert(0, "/opt/trn_rl_repo")

import concourse.bass as bass
import concourse.mybir as mybir
import concourse.tile as tile
from concourse.bass_utils import run_bass_kernel_spmd

# ----------------------------------------------------------------------------
# problem constants (hardcoded per contract)
B, T, D = 32, 4096, 1024
H, KVH, HD = 16, 8, 64
KVD = KVH * HD           # 512
DH4 = 4 * D              # 4096 (MLP hidden)
EPS = 1e-5
NCORES = 8
BL = B // NCORES         # 4 batch rows per core
P = 128
DC = D // P              # 8 d-chunks
TT = 512                 # t-tile
HID_L = DH4 // NCORES    # 512 hidden per core

f32 = mybir.dt.float32
f32r = mybir.dt.float32r
AF = mybir.ActivationFunctionType

# ----------------------------------------------------------------------------
# workaround: this container's walrus accepts at most ONE embedded sync-wait
# per instruction; split extras into standalone single-wait NOPs.


def _legalize_waits_module(module):
    new_module = copy.replace(module, functions=[])
    nfix = 0
    for function in module.functions:
        new_function = copy.replace(function, blocks=[])
        new_function.set_allocations_from_list(function.allocations)
        for block in function.blocks:
            out = []
            for inst in block.instructions:
                si = getattr(inst, "sync_info", None)
                if si is not None and len(si.on_wait) > 1:
                    waits = list(si.on_wait)
                    for w in waits[:-1]:
                        out.append(mybir.InstNoOp(
                            name=f"{inst.name}-w{nfix}",
                            engine=inst.engine,
                            sync_info=mybir.SyncInfo(on_wait=[w], on_update=[]),
                            bass_nofuse=True,
                        ))
                        nfix += 1
                    inst.sync_info = mybir.SyncInfo(
                        on_wait=[waits[-1]], on_update=list(si.on_update))
                out.append(inst)
            new_function.blocks.append(copy.replace(block, instructions=out))
        new_module.functions.append(new_function)
    return new_module


def _install_ntff_hook():
    """run_bass_kernel_spmd(trace=True) needs antenv.axon_hooks, absent in
    this image; rebuild it from trn_boot's ctypes shim."""
    if "antenv.axon_hooks" in sys.modules:
        return True
    try:
        if "/root/.axon_site" not in sys.path:
            sys.path.insert(0, "/root/.axon_site")
        from trn_agent_boot.trn_boot import _ntff_profile_via_ctypes
        hook = _ntff_profile_via_ctypes("/opt/axon/libaxon_pjrt.so")
        if hook is None:
            return False
        mod = types.ModuleType("antenv.axon_hooks")
        mod.get_axon_ntff_profile_hook = lambda: hook
        mod.set_axon_ntff_profile_hook = lambda h: None
        import antenv
        sys.modules["antenv.axon_hooks"] = mod
        antenv.axon_hooks = mod
        return True
    except Exception:
        return False


# ----------------------------------------------------------------------------
def build_module(t_loc=T):
    """Build the SPMD Bass module. t_loc < T shrinks the per-row sequence
    length for cheap simulator runs."""
    ntt = (BL * t_loc) // TT         # number of 512-row enc tiles per core
    tt_per_b = t_loc // TT           # tiles per batch row

    nc = bass.Bass("TRN2", target_bir_lowering=False, debug=False,
                   num_devices=NCORES)

    dram = lambda n, s, d, **kw: nc.dram_tensor(n, s, d, **kw).ap()
    enc = dram("enc", [BL * t_loc, D], f32r, kind="ExternalInput")
    decT_loc = dram("decT_loc", [D, BL], f32, kind="ExternalInput")
    decT_full = dram("decT_full", [D, B], f32, kind="ExternalInput")
    dec_full = dram("dec_full", [B, D], f32, kind="ExternalInput")
    WqT = dram("WqT", [D, D], f32, kind="ExternalInput")
    WkRep = dram("WkRep", [D, D], f32, kind="ExternalInput")
    WvT = dram("WvT", [D, KVD], f32, kind="ExternalInput")
    W1Ts = dram("W1Ts", [2 * D, HID_L], f32, kind="ExternalInput")
    W2Ts = dram("W2Ts", [HID_L, D], f32, kind="ExternalInput")
    ln_g = dram("ln_g", [1, D], f32, kind="ExternalInput")
    ln_b = dram("ln_b", [1, D], f32, kind="ExternalInput")
    ident_r = dram("ident_r", [P, P], f32r, kind="ExternalInput")
    ident_f = dram("ident_f", [P, P], f32, kind="ExternalInput")
    # after the AllReduce every core holds the full result; emit all B rows
    # and let the host read core 0's copy.
    out = dram("out", [B, D], f32, kind="ExternalOutput")

    # collective bounce buffers
    ctxT_cc_in = nc.dram_tensor("ctxT_cc_in", [D, BL], f32)
    ctxT_cc_out = nc.dram_tensor("ctxT_cc_out", [NCORES, D, BL], f32,
                                 addr_space="Shared")
    comb_cc_in = nc.dram_tensor("comb_cc_in", [D, B], f32)
    comb_cc_out = nc.dram_tensor("comb_cc_out", [D, B], f32,
                                 addr_space="Shared")

    with tile.TileContext(nc) as tc, ExitStack() as ctx:
        const = ctx.enter_context(tc.tile_pool(name="const", bufs=1))
        sb_ident_r = const.tile([P, P], f32r)
        nc.sync.dma_start(out=sb_ident_r, in_=ident_r)
        sb_ident_f = const.tile([P, P], f32)
        nc.sync.dma_start(out=sb_ident_f, in_=ident_f)
        sb_wvT = const.tile([P, DC, KVD], f32)
        nc.sync.dma_start(out=sb_wvT, in_=WvT.rearrange("(c p) e -> p c e", p=P))

        state = ctx.enter_context(tc.tile_pool(name="state", bufs=1))
        aT_sb = state.tile([P, DC, KVH, 2, BL], f32r)    # a / 8, transposed
        uT_sb = state.tile([P, DC, BL, H], f32)          # u / l, transposed
        ctxT_sb = state.tile([P, DC, BL], f32)           # ctx, c-dim major

        # ------------------------------------------------------------------
        # Phase 1: q = Wq @ dec (transposed), a = WkRep.T @ q / 8
        with ExitStack() as p1:
            wpool = p1.enter_context(tc.tile_pool(name="p1w", bufs=1))
            spool = p1.enter_context(tc.tile_pool(name="p1s", bufs=2))
            qpsum = p1.enter_context(tc.tile_pool(name="p1qp", bufs=2,
                                                  space="PSUM"))
            apsum = p1.enter_context(tc.tile_pool(name="p1ap", bufs=4,
                                                  space="PSUM"))
            sb_decT = spool.tile([P, DC, BL], f32)
            nc.sync.dma_start(out=sb_decT,
                              in_=decT_loc.rearrange("(c p) b -> p c b", p=P))
            sb_wqT = wpool.tile([P, DC, D], f32, tag="w")
            nc.sync.dma_start(out=sb_wqT,
                              in_=WqT.rearrange("(c p) j -> p c j", p=P))
            qT_sb = spool.tile([P, DC, BL], f32)
            for J in range(DC):
                ps = qpsum.tile([P, BL], f32, tag="q")
                for dc in range(DC):
                    nc.tensor.matmul(ps,
                                     sb_wqT[:, dc, J * P:(J + 1) * P],
                                     sb_decT[:, dc, :],
                                     start=(dc == 0), stop=(dc == DC - 1))
                nc.vector.tensor_copy(qT_sb[:, J, :], ps)
            sb_wkrep = wpool.tile([P, DC, D], f32, tag="w2")
            nc.sync.dma_start(out=sb_wkrep,
                              in_=WkRep.rearrange("(c p) d -> p c d", p=P))
            for dc in range(DC):
                aps = apsum.tile([P, KVH, 2, BL], f32, tag="a")
                for J in range(DC):
                    for pp in range(2):
                        nc.tensor.matmul(
                            aps[:, J, pp, :],
                            sb_wkrep[64 * pp:64 * pp + 64, J,
                                     dc * P:(dc + 1) * P],
                            qT_sb[64 * pp:64 * pp + 64, J, :],
                            start=True, stop=True)
                # fold in the 1/sqrt(hd) = 1/8 scale; evict to f32r
                nc.scalar.activation(aT_sb[:, dc, :, :, :], aps, AF.Copy,
                                     scale=0.125)

        # ------------------------------------------------------------------
        # Phase 2: per batch row, stream enc tiles: transpose, scores, exp,
        # u accumulation. Everything fully unrolled.
        mainw = ctx.enter_context(tc.tile_pool(name="mainw", bufs=1))
        sb_w1T = mainw.tile([P, 2 * DC, HID_L], f32, tag="w1")
        sb_w2T = mainw.tile([P, HID_L // P, D], f32, tag="w2")

        with ExitStack() as p2:
            encp = p2.enter_context(tc.tile_pool(name="enc", bufs=2))
            enctp = p2.enter_context(tc.tile_pool(name="encT", bufs=1))
            smallp = p2.enter_context(tc.tile_pool(name="small", bufs=2))
            upool = p2.enter_context(tc.tile_pool(name="u", bufs=2))
            trps = p2.enter_context(tc.tile_pool(name="trps", bufs=2,
                                                 space="PSUM"))
            sps = p2.enter_context(tc.tile_pool(name="sps", bufs=2,
                                                space="PSUM"))
            ups = p2.enter_context(tc.tile_pool(name="ups", bufs=1,
                                                space="PSUM"))
            ptps = p2.enter_context(tc.tile_pool(name="ptps", bufs=1,
                                                 space="PSUM"))

            for b in range(BL):
                u_ps = ups.tile([H, D], f32, tag="u")
                l_acc = smallp.tile([H, tt_per_b], f32, tag="l")
                for ttl in range(tt_per_b):
                    gtile = b * tt_per_b + ttl
                    # load 512 encoder rows as [128, 4, 1024]
                    sb_enc = encp.tile([P, 4, D], f32r, tag="enc")
                    nc.sync.dma_start(
                        out=sb_enc,
                        in_=enc[gtile * TT:(gtile + 1) * TT, :]
                        .rearrange("(s p) d -> p s d", p=P))
                    # transpose tile: encT [128d, dc, 4s, 128t]
                    sb_encT = enctp.tile([P, DC, 4, P], f32r, tag="encT")
                    for dc in range(DC):
                        trp = trps.tile([P, 4, P], f32r, tag="tr")
                        for s in range(4):
                            nc.tensor.transpose(
                                trp[:, s, :],
                                sb_enc[:, s, dc * P:(dc + 1) * P],
                                sb_ident_r)
                        if dc % 2 == 0:
                            nc.vector.tensor_copy(sb_encT[:, dc, :, :], trp)
                        else:
                            nc.scalar.copy(sb_encT[:, dc, :, :], trp)
                    # scores.T [16, 512] accumulated over d-chunks
                    s_ps = sps.tile([H, TT], f32, tag="s")
                    for dc in range(DC):
                        nc.tensor.matmul(s_ps,
                                         aT_sb[:, dc, :, :, b],
                                         sb_encT[:, dc, :, :],
                                         start=(dc == 0), stop=(dc == DC - 1))
                    # p = exp(scores), l partial = rowsum
                    p_sb = smallp.tile([H, TT], f32r, tag="p")
                    nc.scalar.activation(p_sb, s_ps, AF.Exp,
                                         accum_out=l_acc[:, ttl:ttl + 1])
                    # pT [128t, 4s, 16h]
                    pt_ps = ptps.tile([P, 4, H], f32r, tag="pt")
                    for s in range(4):
                        nc.tensor.transpose(pt_ps[:, s, :],
                                            p_sb[:, s * P:(s + 1) * P],
                                            sb_ident_r[0:H, 0:H])
                    pT_sb = smallp.tile([P, 4, H], f32r, tag="pT")
                    nc.vector.tensor_copy(pT_sb, pt_ps)
                    # u += p @ enc   [16, 1024]
                    first, last = (ttl == 0), (ttl == tt_per_b - 1)
                    for dh in range(2):
                        for s in range(4):
                            nc.tensor.matmul(
                                u_ps[:, dh * TT:(dh + 1) * TT],
                                pT_sb[:, s, :],
                                sb_enc[:, s, dh * TT:(dh + 1) * TT],
                                start=(first and s == 0),
                                stop=(last and s == 3))
                # prefetch MLP weights once, late in the stream
                if b == BL - 1:
                    nc.sync.dma_start(
                        out=sb_w1T,
                        in_=W1Ts.rearrange("(c p) e -> p c e", p=P))
                    nc.sync.dma_start(
                        out=sb_w2T,
                        in_=W2Ts.rearrange("(c p) e -> p c e", p=P))
                # normalize: u /= l ; transpose u -> uT
                l_tot = smallp.tile([H, 1], f32, tag="lt")
                nc.vector.reduce_sum(l_tot, l_acc, axis=mybir.AxisListType.X)
                r_tot = smallp.tile([H, 1], f32, tag="rt")
                nc.vector.reciprocal(r_tot, l_tot)
                u_sb = smallp.tile([H, D], f32, tag="usb")
                nc.scalar.activation(u_sb, u_ps, AF.Copy, scale=r_tot)
                ut_ps = ptps.tile([P, DC, H], f32, tag="ut")
                for dc in range(DC):
                    nc.tensor.transpose(ut_ps[:, dc, :],
                                        u_sb[:, dc * P:(dc + 1) * P],
                                        sb_ident_f[0:H, 0:H])
                nc.vector.tensor_copy(uT_sb[:, :, b, :], ut_ps)

            # ctx = Wv_blk @ u  -> ctxT [c-dim, b]  (reuse the scores psum
            # pool: all PSUM banks are budgeted already)
            for J in range(KVH):
                c_ps = sps.tile([64, BL, 2], f32, tag="s")
                for dc in range(DC):
                    nc.tensor.matmul(
                        c_ps,
                        sb_wvT[:, dc, 64 * J:64 * J + 64],
                        uT_sb[:, dc, :, 2 * J:2 * J + 2],
                        start=(dc == 0), stop=(dc == DC - 1))
                for pp in range(2):
                    nc.vector.tensor_copy(
                        ctxT_sb[64 * pp:64 * pp + 64, J, :],
                        c_ps[:, :, pp])

        # ------------------------------------------------------------------
        # Phase 3: AllGather ctx, tensor-parallel MLP, AllReduce, LayerNorm
        with ExitStack() as p3:
            mp = p3.enter_context(tc.tile_pool(name="mlp", bufs=2))
            hps = p3.enter_context(tc.tile_pool(name="hps", bufs=2,
                                                space="PSUM"))
            cps2 = p3.enter_context(tc.tile_pool(name="cps2", bufs=2,
                                                 space="PSUM"))

            nc.sync.dma_start(
                out=ctxT_cc_in.ap().rearrange("(c p) b -> p c b", p=P),
                in_=ctxT_sb)
            nc.gpsimd.collective_compute(
                "AllGather", mybir.AluOpType.bypass,
                replica_groups=[list(range(NCORES))],
                ins=[ctxT_cc_in.ap().opt()],
                outs=[ctxT_cc_out.ap().opt()])

            # hT chunks [128, 32]: rows 0..1024 = ctx (gathered), 1024..2048 = dec
            sb_hT = mp.tile([P, 2 * DC, B], f32, tag="hT")
            # read back core n's gathered ctxT as hT cols [4n:4n+4]
            for n in range(NCORES):
                nc.sync.dma_start(
                    out=sb_hT[:, 0:DC, BL * n:BL * (n + 1)],
                    in_=bass.AP(tensor=ctxT_cc_out,
                                offset=n * D * BL,
                                ap=[[BL, P], [P * BL, DC], [1, BL]]))
            nc.sync.dma_start(
                out=sb_hT[:, DC:2 * DC, :],
                in_=decT_full.rearrange("(c p) b -> p c b", p=P))

            # hidden = silu(h @ W1s.T): hiddenT [512, 32]
            sb_shT = mp.tile([P, HID_L // P, B], f32, tag="shT")
            for hc in range(HID_L // P):
                h_ps = hps.tile([P, B], f32, tag="h")
                for dc2 in range(2 * DC):
                    nc.tensor.matmul(h_ps,
                                     sb_w1T[:, dc2, hc * P:(hc + 1) * P],
                                     sb_hT[:, dc2, :],
                                     start=(dc2 == 0), stop=(dc2 == 2 * DC - 1))
                sg = mp.tile([P, B], f32, tag="sg")
                nc.scalar.activation(sg, h_ps, AF.Sigmoid)
                nc.vector.tensor_mul(sb_shT[:, hc, :], sg, h_ps)

            # combined partial = silu_h @ W2s.T -> combT [1024, 32]
            sb_cT = mp.tile([P, DC, B], f32, tag="cT")
            for oc in range(DC):
                c2_ps = cps2.tile([P, B], f32, tag="c2")
                for hc in range(HID_L // P):
                    nc.tensor.matmul(c2_ps,
                                     sb_w2T[:, hc, oc * P:(oc + 1) * P],
                                     sb_shT[:, hc, :],
                                     start=(hc == 0),
                                     stop=(hc == HID_L // P - 1))
                nc.vector.tensor_copy(sb_cT[:, oc, :], c2_ps)

            nc.sync.dma_start(
                out=comb_cc_in.ap().rearrange("(c p) b -> p c b", p=P),
                in_=sb_cT)
            nc.gpsimd.collective_compute(
                "AllReduce", mybir.AluOpType.add,
                replica_groups=[list(range(NCORES))],
                ins=[comb_cc_in.ap().opt()],
                outs=[comb_cc_out.ap().opt()])
            sb_cT2 = mp.tile([P, DC, B], f32, tag="cT2")
            nc.sync.dma_start(
                out=sb_cT2,
                in_=comb_cc_out.ap().rearrange("(c p) b -> p c b", p=P))

            # transpose combT back to [32, 1024], add residual, LayerNorm
            comb = mp.tile([B, D], f32, tag="comb")
            for oc in range(DC):
                t_ps = cps2.tile([B, P], f32, tag="t")
                nc.tensor.transpose(t_ps, sb_cT2[:, oc, :], sb_ident_f)
                nc.scalar.copy(comb[:, oc * P:(oc + 1) * P], t_ps)
            sb_dec = mp.tile([B, D], f32, tag="dec")
            nc.sync.dma_start(out=sb_dec, in_=dec_full)
            nc.vector.tensor_add(comb, comb, sb_dec)

            # LayerNorm over the free dim
            stats = mp.tile([B, 2, 6], f32, tag="st")
            mv = mp.tile([B, 2], f32, tag="mv")
            for g in range(2):
                nc.vector.bn_stats(out=stats[:, g, :],
                                   in_=comb[:, g * TT:(g + 1) * TT])
            nc.vector.bn_aggr(out=mv, in_=stats)
            eps_t = mp.tile([B, 1], f32, tag="eps")
            nc.vector.memset(eps_t, EPS)
            std = mp.tile([B, 1], f32, tag="std")
            nc.scalar.activation(std, mv[:, 1:2], AF.Sqrt, bias=eps_t)
            rstd = mp.tile([B, 1], f32, tag="rstd")
            nc.vector.reciprocal(rstd, std)
            normed = mp.tile([B, D], f32, tag="normed")
            nc.vector.tensor_scalar(normed, comb, mv[:, 0:1], rstd,
                                    mybir.AluOpType.subtract,
                                    mybir.AluOpType.mult)
            sb_g = mp.tile([B, D], f32, tag="g")
            nc.sync.dma_start(out=sb_g, in_=ln_g.to_broadcast((B, D)))
            sb_b = mp.tile([B, D], f32, tag="b")
            nc.sync.dma_start(out=sb_b, in_=ln_b.to_broadcast((B, D)))
            nc.vector.tensor_mul(normed, normed, sb_g)
            nc.vector.tensor_add(normed, normed, sb_b)

            nc.sync.dma_start(out=out, in_=normed)

    return nc


_NC_CACHE = {}


def _get_module(t_loc=T):
    if t_loc not in _NC_CACHE:
        _NC_CACHE[t_loc] = build_module(t_loc)
    return _NC_CACHE[t_loc]


def make_in_maps(decoder_hidden, encoder_outputs, Wq, Wk, Wv, W1, W2,
                 ln_g, ln_b, t_loc=T):
    dec = np.ascontiguousarray(decoder_hidden, dtype=np.float32)
    enc = np.ascontiguousarray(encoder_outputs, dtype=np.float32)
    WqT = np.ascontiguousarray(Wq.T, dtype=np.float32)
    # WkRep rows j = h*64+r  ->  Wk[(h//2)*64 + r]
    WkRep = np.ascontiguousarray(
        np.tile(Wk.reshape(KVH, 1, HD, D), (1, 2, 1, 1)).reshape(D, D)
    ).astype(np.float32)
    WvT = np.ascontiguousarray(Wv.T, dtype=np.float32)
    decT = np.ascontiguousarray(dec.T, dtype=np.float32)
    ident = np.eye(P, dtype=np.float32)
    g_row = np.ascontiguousarray(ln_g.reshape(1, D), dtype=np.float32)
    b_row = np.ascontiguousarray(ln_b.reshape(1, D), dtype=np.float32)
    in_maps = []
    for c in range(NCORES):
        in_maps.append({
            "enc": np.ascontiguousarray(
                enc[c * BL:(c + 1) * BL, :t_loc, :]).reshape(BL * t_loc, D),
            "decT_loc": np.ascontiguousarray(decT[:, c * BL:(c + 1) * BL]),
            "decT_full": decT,
            "dec_full": dec,
            "WqT": WqT,
            "WkRep": WkRep,
            "WvT": WvT,
            "W1Ts": np.ascontiguousarray(
                W1[c * HID_L:(c + 1) * HID_L, :].T, dtype=np.float32),
            "W2Ts": np.ascontiguousarray(
                W2[:, c * HID_L:(c + 1) * HID_L].T, dtype=np.float32),
            "ln_g": g_row,
            "ln_b": b_row,
            "ident_r": ident,
            "ident_f": ident,
        })
    return in_maps


LAST_EXEC_TIME_NS = None


def kernel(decoder_hidden, encoder_outputs, Wq, Wk, Wv, W1, W2, ln_g, ln_b):
    global LAST_EXEC_TIME_NS
    t_loc = encoder_outputs.shape[1]
    nc = _get_module(t_loc)
    in_maps = make_in_maps(decoder_hidden, encoder_outputs, Wq, Wk, Wv,
                           W1, W2, ln_g, ln_b, t_loc=t_loc)
    trace = bool(int(os.environ.get("BASS_KERNEL_TRACE", "0")))
    if trace:
        _install_ntff_hook()
    if not getattr(nc, "_waits_legalized", False):
        nc.m = _legalize_waits_module(nc.m)
        nc._waits_legalized = True
    res = run_bass_kernel_spmd(nc, in_maps, list(range(NCORES)), trace=trace)
    LAST_EXEC_TIME_NS = res.exec_time_ns
    return np.ascontiguousarray(res.results[0]["out"], dtype=np.float32)
